# revision 1
# baseline (speedup 1.0000x reference)
"""Trainium2 Bass kernel for nn_AttentionResidualBlock (B=16, C=256, H=W=32, heads=8).

Sharding: data-parallel over batch across 8 NeuronCores (2 images/core),
weights replicated; attention heads processed in pairs on-chip.

Per core:
  - conv3x3 as 9 shifted bf16 matmuls over a zero-padded [C, 34, 34] layout;
    BN scale folded into weights on host, BN shift + ReLU fused on DVE.
    Conv work is interleaved into the attention pairs as TensorE filler.
  - attention: scoresT[m,n] = k^T q per head; the two heads of a pair are
    issued back-to-back on different PE row-groups (K=32 row tiling) into
    one PSUM tile; exp on ScalarE (PSUM -> SBUF bf16); attention output +
    softmax row-sum in one accumulation via a ones-augmented v (M=33,
    col positions 0/64 for the pair); denominators applied via DVE
    reciprocal + broadcast DMA (through a DRAM scratch) + multiply.
  - out-projection consumes the on-chip head layout directly; out_w is
    row-permuted (and gate/out-bias/v-bias folded) on host.
All matmuls bf16 with fp32 PSUM accumulation (~2.2e-3 scale-rel err vs fp32).
"""

import numpy as np
import ml_dtypes
from contextlib import ExitStack

import concourse.bass as bass
import concourse.bacc as bacc
import concourse.mybir as mybir
import concourse.tile as tile
from concourse.bass_utils import run_bass_kernel_spmd

F32 = mybir.dt.float32
BF16 = mybir.dt.bfloat16
AF = mybir.ActivationFunctionType
ALU = mybir.AluOpType

C = 256
HEADS = 8
D = 32
B, H, W = 16, 32, 32
N = H * W          # 1024
HP = H + 2         # 34
EPS = 1e-5
N_CORES = 8
IMGS = B // N_CORES  # 2 images per core
CC = C // 128      # 2 channel chunks
MC = N // 128      # 8 spatial m-chunks
DAUG = D + 1       # 33 (v rows + ones row)

# packed bf16 weight layout (columns per partition)
W1_COLS = CC * 9 * CC * 128          # 4608
QKVO_COLS = CC * C                   # 512
PACK_COLS = 2 * W1_COLS + 3 * QKVO_COLS + 4 * C  # w1 w2 q k v ow


def _bcast_ap(sliced: bass.AP, parts: int) -> bass.AP:
    """Broadcast a DRAM [F] AP to [parts, F] (prepended step-0 dim)."""
    ap = [[0, parts]] + [list(d) for d in sliced.ap]
    return bass.AP(tensor=sliced.tensor, offset=sliced.offset, ap=ap)


def build_nc() -> bass.Bass:
    nc = bacc.Bacc()

    x_d = nc.declare_dram_parameter("x_sh", [IMGS, CC, 128, N], F32, isOutput=False)
    wp_d = nc.declare_dram_parameter("wpack", [128, PACK_COLS], BF16, isOutput=False)
    vec_d = nc.declare_dram_parameter("vecs", [128, 3 * CC], F32, isOutput=False)
    out_d = nc.declare_dram_parameter("out_sh", [IMGS, CC, 128, N], F32, isOutput=True)

    with ExitStack() as ctx:
        tc = ctx.enter_context(tile.TileContext(nc))
        wpool = ctx.enter_context(tc.tile_pool(name="weights", bufs=1))
        xpool = ctx.enter_context(tc.tile_pool(name="acts", bufs=2))
        ptpool = ctx.enter_context(tc.tile_pool(name="pt", bufs=2))
        ps_sc = ctx.enter_context(tc.tile_pool(name="ps_sc", bufs=2, space="PSUM"))
        ps_at = ctx.enter_context(tc.tile_pool(name="ps_at", bufs=1, space="PSUM"))
        ps_cv = ctx.enter_context(tc.tile_pool(name="ps_cv", bufs=1, space="PSUM"))
        dpool = ctx.enter_context(tc.tile_pool(name="dram", bufs=2, space="DRAM"))

        # ---- x loads first (the first compute consumer), then weights ----
        xtiles = {}

        def xload(img):
            xpad = xpool.tile([128, CC, HP, HP], F32, tag="xpad", name="xpad")
            xpadb = xpool.tile([128, CC, HP, HP], BF16, tag="xpadb", name="xpadb")
            for cc in range(CC):
                nc.vector.memset(xpad[:, cc], 0.0)
                nc.sync.dma_start(
                    out=xpad[:, cc, 1:HP - 1, 1:HP - 1],
                    in_=x_d[img, cc].rearrange("p (r c) -> p r c", r=H))
                nc.vector.tensor_copy(xpadb[:, cc], xpad[:, cc])
            xtiles[img] = (xpad, xpadb)

        xload(0)
        # qkv weights + vecs first (first consumers); the big conv-weight
        # transfer last — it is not needed until the first conv filler
        wpack = wpool.tile([128, PACK_COLS], BF16, tag="wpack")
        nc.sync.dma_start(out=wpack[:, 2 * W1_COLS:], in_=wp_d[:, 2 * W1_COLS:])
        vecs = wpool.tile([128, 3 * CC], F32, tag="vecs")
        nc.sync.dma_start(out=vecs, in_=vec_d[:])
        nc.sync.dma_start(out=wpack[:, :2 * W1_COLS], in_=wp_d[:, :2 * W1_COLS])
        o_w1, o_w2 = 0, W1_COLS
        o_q = 2 * W1_COLS
        o_k, o_v = o_q + QKVO_COLS, o_q + 2 * QKVO_COLS
        o_ow = o_q + 3 * QKVO_COLS

        def conv_w(base, ic, tap, oc):  # [128, 128] lhsT slice
            off = base + ((ic * 9 + tap) * CC + oc) * 128
            return wpack[:, off:off + 128]

        shift1 = lambda oc: vecs[:, oc:oc + 1]
        shiftF = lambda oc: vecs[:, CC + oc:CC + oc + 1]
        qbias = lambda oc: vecs[:, 2 * CC + oc:2 * CC + oc + 1]
        rs_tmp = wpool.tile([128, N], F32, tag="rstmp")
        bcast = wpool.tile([128, N], F32, tag="bcast")
        cmb1 = wpool.tile([128, N], F32, tag="cmb1")

        xload(1)

        def xflat(t, cc):  # unpadded [p, 32, 32] view
            return t[:, cc, 1:HP - 1, 1:HP - 1]

        # ---- qkv helpers (img1's chunks are prefetched into img0's pairs,
        #      borrowing the at-pool psum slot between normalize and the
        #      next pair's attention) ----
        qkv_tiles = {}

        def qkv_alloc(img):
            xpadb = xtiles[img][1]
            d = {
                "q": xpool.tile([128, CC, N], BF16, tag="q", name="q_sb"),
                "k": xpool.tile([128, CC, N], BF16, tag="k", name="k_sb"),
                "xnb": xpool.tile([128, CC, N], BF16, tag="xnb", name="xnb"),
                "vaug": xpool.tile([128, MC, HEADS, DAUG], BF16, tag="vaug",
                                   name="v_aug"),
            }
            for cc in range(CC):
                nc.vector.tensor_copy(
                    d["xnb"][:, cc].rearrange("p (r c) -> p r c", r=H),
                    xflat(xpadb, cc))
            for mc in range(MC):
                nc.vector.memset(d["vaug"][:, mc], 1.0)
            qkv_tiles[img] = d
            return d

        def qk_chunk(img, oc, which, pool, tag):
            d = qkv_tiles[img]
            xpadb = xtiles[img][1]
            wb = o_q if which == "q" else o_k
            ps = pool.tile([128, N], F32, tag=tag, name=f"ps{which}{oc}")
            for nh in range(2):
                for ic in range(CC):
                    nc.tensor.matmul(
                        ps[:, nh * 512:(nh + 1) * 512],
                        lhsT=wpack[:, wb + ic * C + oc * 128:
                                   wb + ic * C + (oc + 1) * 128],
                        rhs=xflat(xpadb, ic)[:, nh * 16:(nh + 1) * 16, :],
                        start=(ic == 0), stop=(ic == CC - 1))
            if which == "q":
                nc.vector.tensor_scalar(d["q"][:, oc], ps, qbias(oc), None, ALU.add)
            else:
                nc.vector.tensor_copy(d["k"][:, oc], ps)

        def v_chunk(img, half, pool, tag):
            d = qkv_tiles[img]
            ps = pool.tile([128, N], F32, tag=tag, name=f"psv{half}")
            for mcl in range(4):
                mc = half * 4 + mcl
                for ic in range(CC):
                    nc.tensor.matmul(
                        ps[:, mcl * C:(mcl + 1) * C],
                        lhsT=d["xnb"][:, ic, mc * 128:(mc + 1) * 128],
                        rhs=wpack[:, o_v + ic * C: o_v + (ic + 1) * C],
                        start=(ic == 0), stop=(ic == CC - 1))
            for mcl in range(4):
                mc = half * 4 + mcl
                nc.vector.tensor_copy(
                    d["vaug"][:, mc, :, 0:D],
                    ps[:, mcl * C:(mcl + 1) * C].rearrange("p (h e) -> p h e", h=HEADS))

        def qkv_full(img):
            # q/k only: scores need them first; v is emitted inside pair 0
            # (attention needs it only at attn_block time)
            qkv_alloc(img)
            for oc in range(CC):
                qk_chunk(img, oc, "q", ps_sc, "sc")
                qk_chunk(img, oc, "k", ps_sc, "sc")

        for img in range(IMGS):
            xpad, xpadb = xtiles[img]
            if img not in qkv_tiles:
                qkv_full(img)
            d_qkv = qkv_tiles[img]
            q_sb, k_sb, v_aug = d_qkv["q"], d_qkv["k"], d_qkv["vaug"]

            # ---- conv pieces (PE filler inside attention pairs) ----
            c1pad = xpool.tile([128, CC, HP, HP], BF16, tag="c1pad")
            for cc in range(CC):
                nc.vector.memset(c1pad[:, cc], 0.0)
            c2x = xpool.tile([128, CC, N], F32, tag="c2x")  # conv2 + shiftF + x

            conv_ps = {}

            def conv_half(key, w_base, src_pad, oc, nh):
                if key not in conv_ps:
                    conv_ps[key] = ps_cv.tile([128, N], F32, tag="cv", name=key)
                ps = conv_ps[key]
                first = True
                for ic in range(CC):
                    for tap in range(9):
                        ky, kx = divmod(tap, 3)
                        nc.tensor.matmul(
                            ps[:, nh * 512:(nh + 1) * 512],
                            lhsT=conv_w(w_base, ic, tap, oc),
                            rhs=src_pad[:, ic, ky + nh * 16:ky + nh * 16 + 16, kx:kx + W],
                            start=first, stop=(ic == CC - 1 and tap == 8))
                        first = False
                return ps

            def c1_work(oc, nh):
                ps = conv_half(f"c1{oc}", o_w1, xpadb, oc, nh)
                nc.vector.tensor_scalar(
                    xflat(c1pad, oc)[:, nh * 16:(nh + 1) * 16, :],
                    ps.rearrange("p (r c) -> p r c", r=H)[:, nh * 16:(nh + 1) * 16, :],
                    shift1(oc), 0.0, ALU.add, ALU.max)
                if nh == 1:
                    del conv_ps[f"c1{oc}"]

            def c2_work(oc, nh):
                ps = conv_half(f"c2{oc}", o_w2, c1pad, oc, nh)
                nc.vector.scalar_tensor_tensor(
                    out=c2x[:, oc].rearrange("p (r c) -> p r c", r=H)[:, nh * 16:(nh + 1) * 16, :],
                    in0=ps.rearrange("p (r c) -> p r c", r=H)[:, nh * 16:(nh + 1) * 16, :],
                    scalar=shiftF(oc),
                    in1=xflat(xpad, oc)[:, nh * 16:(nh + 1) * 16, :],
                    op0=ALU.add, op1=ALU.add)
                if nh == 1:
                    del conv_ps[f"c2{oc}"]

            fillers = [
                [lambda: c1_work(0, 0), lambda: c1_work(0, 1)],
                [lambda: c1_work(1, 0), lambda: c1_work(1, 1)],
                [lambda: c2_work(0, 0), lambda: c2_work(0, 1)],
                [lambda: c2_work(1, 0), lambda: c2_work(1, 1)],
            ]

            # ---- attention: head pairs ----
            attn_t = [xpool.tile([128, N], BF16, tag=f"attn{p_}", name=f"attn{p_}")
                      for p_ in range(4)]
            for p_ in range(4):
                nc.vector.memset(attn_t[p_], 0.0)
            rs_dr = dpool.tile([4, 2, N], F32, tag="rsdram", name="rs_dr")

            # software-pipelined pairs: the next pair's first-half scores
            # are emitted before this pair's attention, so ScalarE's exp
            # stream never drains while PE runs attention + conv filler
            pair_pt = {}

            def scores_half(p_, half):
                pt = pair_pt.setdefault(p_, {})
                ha, hb = 2 * p_, 2 * p_ + 1
                hpa, cca = 32 * (ha % 4), ha // 4
                hpb, ccb = 32 * (hb % 4), hb // 4
                for mc in range(half * 4, half * 4 + 4):
                    for nh in range(2):
                        # one PSUM tile holds this n-half for both heads; the
                        # two K=32 matmuls sit on different PE row groups
                        sc = ps_sc.tile([128, N], F32, tag="sc", name="sc")
                        nsl = slice(nh * 512, (nh + 1) * 512)
                        nc.tensor.matmul(
                            sc[:, 0:512],
                            lhsT=k_sb[hpa:hpa + 32, cca, mc * 128:(mc + 1) * 128],
                            rhs=q_sb[hpa:hpa + 32, cca, nsl],
                            start=True, stop=True, tile_position=(hpa, 0))
                        nc.tensor.matmul(
                            sc[:, 512:1024],
                            lhsT=k_sb[hpb:hpb + 32, ccb, mc * 128:(mc + 1) * 128],
                            rhs=q_sb[hpb:hpb + 32, ccb, nsl],
                            start=True, stop=True, tile_position=(hpb, 0))
                        p = ptpool.tile([128, N], BF16, tag=f"pt{mc}_{nh}",
                                        name=f"pt{mc}_{nh}")
                        nc.scalar.activation(p, sc, AF.Exp)
                        pt[mc, nh] = p
                fillers[p_].pop(0)()

            def attn_block(p_):
                pt = pair_pt[p_]
                ha, hb = 2 * p_, 2 * p_ + 1
                # attn + rowsum into one [128, N] tile: head a rows 0:33
                # (PE col strips 0-1), head b rows 64:97 (strips 2-3) —
                # interleaved issue so the two M=33 matmuls overlap in the
                # array; each head runs its own accumulation group with its
                # own start=True (HW-verified safe on shared banks)
                at = ps_at.tile([128, N], F32, tag="at", name="at")
                for mc in range(MC):
                    for nh in range(2):
                        nsl = slice(nh * 512, (nh + 1) * 512)
                        for i, h in ((0, ha), (1, hb)):
                            nc.tensor.matmul(
                                at[64 * i:64 * i + DAUG, nsl],
                                lhsT=v_aug[:, mc, h, :],
                                rhs=pt[mc, nh][:, 512 * i:512 * i + 512],
                                start=(mc == 0), stop=(mc == MC - 1))

                # normalize: 1/rowsum (rows 32 / 96), broadcast, multiply
                for i in range(2):
                    r = 64 * i + D
                    nc.vector.reciprocal(rs_tmp[r:r + 1], at[r:r + 1])
                for i in range(2):
                    r = 64 * i + D
                    nc.sync.dma_start(out=rs_dr[p_, i], in_=rs_tmp[r:r + 1])
                    nc.sync.dma_start(
                        out=bcast[64 * i:64 * i + D],
                        in_=_bcast_ap(rs_dr[p_, i], D))
                for i in range(2):
                    nc.vector.tensor_tensor(
                        attn_t[p_][64 * i:64 * i + D],
                        at[64 * i:64 * i + D],
                        bcast[64 * i:64 * i + D],
                        ALU.mult)

                # prefetch next image's qkv in the at-slot gap
                if img == 0:
                    if p_ == 0:
                        qkv_alloc(1)
                        qk_chunk(1, 0, "q", ps_at, "at")
                        qk_chunk(1, 1, "q", ps_at, "at")
                    elif p_ == 1:
                        qk_chunk(1, 0, "k", ps_at, "at")
                        qk_chunk(1, 1, "k", ps_at, "at")
                    elif p_ == 2:
                        v_chunk(1, 0, ps_at, "at")
                    else:
                        v_chunk(1, 1, ps_at, "at")
                        qkv_tiles[1]["v_done"] = True

            scores_half(0, 0)
            if not d_qkv.get("v_done"):
                v_chunk(img, 0, ps_at, "at")
                v_chunk(img, 1, ps_at, "at")
                d_qkv["v_done"] = True
            scores_half(0, 1)
            for p_ in range(4):
                if p_ < 3:
                    scores_half(p_ + 1, 0)
                attn_block(p_)
                if p_ < 3:
                    scores_half(p_ + 1, 1)

            # ---- proj + combine ----
            pj = {0: ps_cv.tile([128, N], F32, tag="cv", name="pj0"),
                  1: ps_sc.tile([128, N], F32, tag="sc", name="pj1")}
            def pj_mm(oc, nh, kc):
                nc.tensor.matmul(
                    pj[oc][:, nh * 512:(nh + 1) * 512],
                    lhsT=wpack[:, o_ow + kc * C + oc * 128:
                               o_ow + kc * C + (oc + 1) * 128],
                    rhs=attn_t[kc][:, nh * 512:(nh + 1) * 512],
                    start=(kc == 0), stop=(kc == 3))

            # kc 0..2 only need the first three pairs' attention output, so
            # they overlap the last pair's normalize; kc3 closes each group
            for oc in range(CC):
                for nh in range(2):
                    for kc in range(3):
                        pj_mm(oc, nh, kc)
            for oc in range(CC):
                for nh in range(2):
                    pj_mm(oc, nh, 3)
                out_sb = xpool.tile([128, N], F32, tag="out", name="out_sb")
                nc.vector.tensor_tensor(cmb1, c2x[:, oc], pj[oc], ALU.add)
                nc.vector.tensor_scalar(out_sb, cmb1, 0.0, None, ALU.max)
                nc.sync.dma_start(out=out_d[img, oc], in_=out_sb)

    nc.finalize()
    return nc


def _prep_inputs(inputs: dict) -> list[dict]:
    bf = ml_dtypes.bfloat16
    x = np.asarray(inputs["x"], dtype=np.float32)
    f32 = lambda k: np.asarray(inputs[k], dtype=np.float32)
    bn1_inv = f32("bn1_gamma") / np.sqrt(f32("bn1_var") + EPS)
    shift1 = f32("bn1_beta") - f32("bn1_mean") * bn1_inv + f32("conv1_b") * bn1_inv
    w1s = f32("conv1_w") * bn1_inv[:, None, None, None]
    bn2_inv = f32("bn2_gamma") / np.sqrt(f32("bn2_var") + EPS)
    shift2 = f32("bn2_beta") - f32("bn2_mean") * bn2_inv + f32("conv2_b") * bn2_inv
    w2s = f32("conv2_w") * bn2_inv[:, None, None, None]
    sg = 1.0 / (1.0 + np.exp(-float(np.asarray(inputs["gate"]))))
    ow = f32("out_w") * sg
    shiftF = shift2 + sg * f32("out_b") + sg * (f32("out_w") @ f32("v_b"))
    qws = f32("q_w") / np.sqrt(D)
    qbs = f32("q_b") / np.sqrt(D)

    def conv_pack(w):  # [O, I, 3, 3] -> [128, CC*9*CC*128]
        t = w.transpose(1, 2, 3, 0).reshape(CC, 128, 3, 3, CC, 128)
        return t.transpose(1, 0, 2, 3, 4, 5).reshape(128, W1_COLS)

    def pack_T(w):  # [O, C_in] -> [128, CC*C]
        return w.T.reshape(CC, 128, C).transpose(1, 0, 2).reshape(128, QKVO_COLS)

    owT = ow.T  # [C_in, C_out]
    owp = np.zeros((4, 128, C), np.float32)
    for p_ in range(4):
        owp[p_, 0:32] = owT[64 * p_: 64 * p_ + 32]
        owp[p_, 64:96] = owT[64 * p_ + 32: 64 * p_ + 64]
    owpk = owp.transpose(1, 0, 2).reshape(128, 4 * C)

    wpack = np.concatenate(
        [conv_pack(w1s), conv_pack(w2s), pack_T(qws), pack_T(f32("k_w")),
         pack_T(f32("v_w")), owpk], axis=1).astype(bf)
    assert wpack.shape == (128, PACK_COLS)

    vecs = np.stack([shift1.reshape(CC, 128), shiftF.reshape(CC, 128),
                     qbs.reshape(CC, 128)]).reshape(3 * CC, 128).T
    shared = {"wpack": np.ascontiguousarray(wpack),
              "vecs": np.ascontiguousarray(vecs.astype(np.float32))}
    in_maps = []
    for core in range(N_CORES):
        xs = x[core * IMGS:(core + 1) * IMGS].reshape(IMGS, CC, 128, N)
        in_maps.append({"x_sh": np.ascontiguousarray(xs), **shared})
    return in_maps


_NC_CACHE = {}


def _get_nc():
    if "nc" not in _NC_CACHE:
        _NC_CACHE["nc"] = build_nc()
    return _NC_CACHE["nc"]


def kernel(**inputs) -> np.ndarray:
    nc = _get_nc()
    in_maps = _prep_inputs(inputs)
    res = run_bass_kernel_spmd(nc, in_maps, core_ids=list(range(N_CORES)))
    outs = [res.results[i]["out_sh"].reshape(IMGS, C, H, W) for i in range(N_CORES)]
    return np.concatenate(outs, axis=0)



# revision 10
# speedup vs baseline: 1.2381x; 1.2381x over previous
"""Trainium2 Bass kernel for nn_AttentionResidualBlock (B=16, C=256, H=W=32, heads=8).

Sharding: data-parallel over batch across 8 NeuronCores (2 images/core),
weights replicated.

Per core (per image):
  - conv3x3 as 9 shifted bf16 matmuls over a zero-padded [C, 34, 34] layout;
    BN scale folded into weights on host, BN shift + ReLU fused on DVE.
    Conv work paces as TensorE filler inside the attention slots.
  - attention head-by-head: scoresT[m,n] = k^T q (K=32 row tiles); exp on
    ScalarE (PSUM -> SBUF bf16); attn@v computed TRANSPOSED:
    out[n, d] = sum_m pt[m, n] v[m, d] with lhsT = pt blocks and a
    ones-augmented v (33 cols) so the softmax denominator lands on the
    same partition as its outputs. Normalize is then a per-partition
    reciprocal + tensor_scalar multiply (no cross-partition broadcast).
  - attnT is transposed back to [c, n] with PE transposes (f32), then a
    dense out-projection; gate/out-bias/v-bias folded on host.
All matmuls bf16 with fp32 PSUM accumulation.
"""

import numpy as np
import ml_dtypes
from contextlib import ExitStack

import concourse.bass as bass
import concourse.bacc as bacc
import concourse.mybir as mybir
import concourse.tile as tile
from concourse.bass_utils import run_bass_kernel_spmd

F32 = mybir.dt.float32
BF16 = mybir.dt.bfloat16
AF = mybir.ActivationFunctionType
ALU = mybir.AluOpType

C = 256
HEADS = 8
D = 32
B, H, W = 16, 32, 32
N = H * W          # 1024
HP = H + 2         # 34
EPS = 1e-5
N_CORES = 8
IMGS = B // N_CORES  # 2 images per core
CC = C // 128      # 2 channel chunks
MC = N // 128      # 8 spatial m-chunks
NCH = 8            # n-chunks for attn output
DAUG = D + 1       # 33 (v cols + ones col)

# packed bf16 weight layout (columns per partition)
W1_COLS = CC * 9 * CC * 128          # 4608
QKVO_COLS = CC * C                   # 512
PACK_COLS = 2 * W1_COLS + 4 * QKVO_COLS  # w1 w2 q k v ow = 11264
VEC_COLS = 3 * CC + 128              # shift1, shiftF, qbias, identity(f32)


def build_nc() -> bass.Bass:
    nc = bacc.Bacc()

    x_d = nc.declare_dram_parameter("x_sh", [IMGS, CC, 128, N], F32, isOutput=False)
    wp_d = nc.declare_dram_parameter("wpack", [128, PACK_COLS], BF16, isOutput=False)
    vec_d = nc.declare_dram_parameter("vecs", [128, VEC_COLS], F32, isOutput=False)
    out_d = nc.declare_dram_parameter("out_sh", [IMGS, CC, 128, N], F32, isOutput=True)

    o_w1, o_w2 = 0, W1_COLS
    o_q = 2 * W1_COLS
    o_k, o_v = o_q + QKVO_COLS, o_q + 2 * QKVO_COLS
    o_ow = o_q + 3 * QKVO_COLS

    with ExitStack() as ctx:
        tc = ctx.enter_context(tile.TileContext(nc))
        wpool = ctx.enter_context(tc.tile_pool(name="weights", bufs=1))
        xpool = ctx.enter_context(tc.tile_pool(name="acts", bufs=2))
        ptpool = ctx.enter_context(tc.tile_pool(name="pt", bufs=18))
        ps_sc = ctx.enter_context(tc.tile_pool(name="ps_sc", bufs=2, space="PSUM"))
        ps_at = ctx.enter_context(tc.tile_pool(name="ps_at", bufs=2, space="PSUM"))
        ps_cv = ctx.enter_context(tc.tile_pool(name="ps_cv", bufs=2, space="PSUM"))

        # ---- weights / vectors ----
        wpack = wpool.tile([128, PACK_COLS], BF16, tag="wpack")
        vecs = wpool.tile([128, VEC_COLS], F32, tag="vecs")

        def conv_w(base, ic, tap, oc):  # [128, 128] lhsT slice
            off = base + ((ic * 9 + tap) * CC + oc) * 128
            return wpack[:, off:off + 128]

        shift1 = lambda oc: vecs[:, oc:oc + 1]
        shiftF = lambda oc: vecs[:, CC + oc:CC + oc + 1]
        qbias = lambda oc: vecs[:, 2 * CC + oc:2 * CC + oc + 1]
        ident = vecs[:, 3 * CC:3 * CC + 128]

        # ---- filler queue (PE work units paced into attention slots) ----
        queue = []

        def push(cycles, fn, front=False):
            if front:
                queue.insert(0, (cycles, fn))
            else:
                queue.append((cycles, fn))

        def pop_fill(budget=1400):
            done = 0
            while queue and done < budget:
                cyc, fn = queue.pop(0)
                fn()
                done += cyc

        def drain_queue():
            while queue:
                _, fn = queue.pop(0)
                fn()

        # ---- per-image tiles ----
        xtiles = {}

        def xload_dma(img):
            xpad = xpool.tile([128, CC, HP, HP], F32, tag="xpad", name=f"xpad{img}")
            xpadb = xpool.tile([128, CC, HP, HP], BF16, tag="xpadb", name=f"xpadb{img}")
            for cc in range(CC):
                nc.vector.memset(xpad[:, cc, 0, :], 0.0)
                nc.vector.memset(xpad[:, cc, HP - 1, :], 0.0)
                nc.vector.memset(xpad[:, cc, 1:HP - 1, 0], 0.0)
                nc.vector.memset(xpad[:, cc, 1:HP - 1, HP - 1], 0.0)
                nc.sync.dma_start(
                    out=xpad[:, cc, 1:HP - 1, 1:HP - 1],
                    in_=x_d[img, cc].rearrange("p (r c) -> p r c", r=H))
            xtiles[img] = (xpad, xpadb)

        def xcast(img):
            xpad, xpadb = xtiles[img]
            for cc in range(CC):
                nc.vector.tensor_copy(xpadb[:, cc], xpad[:, cc])

        def xflat(t, cc):  # unpadded [p, 32, 32] view
            return t[:, cc, 1:HP - 1, 1:HP - 1]

        # ---- qkv ----
        qkv_tiles = {}

        def qkv_alloc(img):
            d = {
                "q": xpool.tile([128, CC, N], BF16, tag="q", name=f"q{img}"),
                "k": xpool.tile([128, CC, N], BF16, tag="k", name=f"k{img}"),
                "xnb": xpool.tile([128, CC, N], BF16, tag="xnb", name=f"xnb{img}"),
                "vaug": xpool.tile([128, MC, HEADS, DAUG], BF16, tag="vaug",
                                   name=f"vaug{img}"),
                "attnT": xpool.tile([128, NCH, C], BF16, tag="attnT",
                                    name=f"attnT{img}"),
                "A": xpool.tile([128, CC, N], BF16, tag="A", name=f"A{img}"),
            }
            xpadb = xtiles[img][1]
            for cc in range(CC):
                nc.vector.tensor_copy(
                    d["xnb"][:, cc].rearrange("p (r c) -> p r c", r=H),
                    xflat(xpadb, cc))
            qkv_tiles[img] = d
            return d

        def qk_chunk(img, oc, which):
            d = qkv_tiles[img]
            xpadb = xtiles[img][1]
            wb = o_q if which == "q" else o_k
            ps = ps_sc.tile([128, N], F32, tag="sc", name=f"ps{which}{img}_{oc}")
            for nh in range(2):
                for ic in range(CC):
                    nc.tensor.matmul(
                        ps[:, nh * 512:(nh + 1) * 512],
                        lhsT=wpack[:, wb + ic * C + oc * 128:
                                   wb + ic * C + (oc + 1) * 128],
                        rhs=xflat(xpadb, ic)[:, nh * 16:(nh + 1) * 16, :],
                        start=(ic == 0), stop=(ic == CC - 1))
            if which == "q":
                nc.vector.tensor_scalar(d["q"][:, oc], ps, qbias(oc), None, ALU.add)
            else:
                nc.vector.tensor_copy(d["k"][:, oc], ps)

        def v_chunk(img, half):
            d = qkv_tiles[img]
            xpadb = xtiles[img][1]
            if half == 0:
                nc.vector.memset(d["vaug"][:, :, :, D:DAUG], 1.0)
            ps = ps_sc.tile([128, N], F32, tag="sc", name=f"psv{img}_{half}")
            for mcl in range(4):
                mc = half * 4 + mcl
                for ic in range(CC):
                    nc.tensor.matmul(
                        ps[:, mcl * C:(mcl + 1) * C],
                        lhsT=d["xnb"][:, ic, mc * 128:(mc + 1) * 128],
                        rhs=wpack[:, o_v + ic * C: o_v + (ic + 1) * C],
                        start=(ic == 0), stop=(ic == CC - 1))
            for mcl in range(4):
                mc = half * 4 + mcl
                nc.vector.tensor_copy(
                    d["vaug"][:, mc, :, 0:D],
                    ps[:, mcl * C:(mcl + 1) * C].rearrange("p (h e) -> p h e", h=HEADS))

        # ---- conv chains (filler units) ----
        def push_conv_units(img, cname, w_base, oc, nh):
            state = {}
            mmlist = [(ic, tap) for ic in range(CC) for tap in range(9)]

            def consume(ps):
                xpad, xpadb = xtiles[img]
                if cname == "c1":
                    nc.vector.tensor_scalar(
                        xflat(c1pads[img], oc)[:, nh * 16:(nh + 1) * 16, :],
                        ps.rearrange("p (r c) -> p r c", r=16),
                        shift1(oc), 0.0, ALU.add, ALU.max)
                else:
                    nc.vector.scalar_tensor_tensor(
                        out=c2xs[img][:, oc, nh * 512:(nh + 1) * 512]
                            .rearrange("p (r c) -> p r c", r=16),
                        in0=ps.rearrange("p (r c) -> p r c", r=16),
                        scalar=shiftF(oc),
                        in1=xflat(xpad, oc)[:, nh * 16:(nh + 1) * 16, :],
                        op0=ALU.add, op1=ALU.add)

            def mk(i0, i1):
                def fn():
                    if "ps" not in state:
                        state["ps"] = ps_cv.tile([128, 512], F32, tag="cv",
                                                 name=f"{cname}{img}_{oc}_{nh}")
                    ps = state["ps"]
                    src = xtiles[img][1] if cname == "c1" else c1pads[img]
                    for idx in range(i0, i1):
                        ic, tap = mmlist[idx]
                        ky, kx = divmod(tap, 3)
                        nc.tensor.matmul(
                            ps,
                            lhsT=conv_w(w_base, ic, tap, oc),
                            rhs=src[:, ic, ky + nh * 16:ky + nh * 16 + 16, kx:kx + W],
                            start=(idx == 0), stop=(idx == 17))
                    if i1 == 18:
                        consume(ps)
                return fn

            for i0 in range(0, 18, 3):
                push(3 * 512, mk(i0, min(i0 + 3, 18)))

        c1pads, c2xs = {}, {}

        def conv_alloc(img):
            c1pad = xpool.tile([128, CC, HP, HP], BF16, tag="c1pad", name=f"c1p{img}")
            for cc in range(CC):
                nc.vector.memset(c1pad[:, cc, 0, :], 0.0)
                nc.vector.memset(c1pad[:, cc, HP - 1, :], 0.0)
                nc.vector.memset(c1pad[:, cc, 1:HP - 1, 0], 0.0)
                nc.vector.memset(c1pad[:, cc, 1:HP - 1, HP - 1], 0.0)
            c1pads[img] = c1pad
            c2xs[img] = xpool.tile([128, CC, N], F32, tag="c2x", name=f"c2x{img}")

        def push_conv_all(img):
            for oc in range(CC):
                for nh in range(2):
                    push_conv_units(img, "c1", o_w1, oc, nh)
            for oc in range(CC):
                for nh in range(2):
                    push_conv_units(img, "c2", o_w2, oc, nh)

        # ---- attention ----
        def emit_head_norm(img, h, at):
            d = qkv_tiles[img]
            rcp = xpool.tile([128, NCH], F32, tag="rcp", name=f"rcp{img}_{h}")
            nc.vector.reciprocal(
                rcp, at.rearrange("p (g e) -> p g e", e=DAUG)[:, :, D])
            for ncb in range(NCH):
                nc.vector.tensor_scalar(
                    d["attnT"][:, ncb, h * D:(h + 1) * D],
                    at[:, ncb * DAUG:ncb * DAUG + D],
                    rcp[:, ncb:ncb + 1], None, ALU.mult)

        def push_transp(img, cc):
            # DMA XBAR transpose: SBUF->SBUF, no PSUM, runs on DMA engines
            d = qkv_tiles[img]
            for ncb in range(NCH):
                def fn(ncb=ncb):
                    nc.sync.dma_start_transpose(
                        out=d["A"][:, cc, ncb * 128:(ncb + 1) * 128],
                        in_=d["attnT"][:, ncb, cc * 128:(cc + 1) * 128])
                push(0, fn, front=(ncb < 4))

        def push_proj(img):
            d = qkv_tiles[img]
            for oc in range(CC):
                for nh in range(2):
                    def fn(oc=oc, nh=nh):
                        pj = ps_cv.tile([128, 512], F32, tag="cv",
                                        name=f"pj{img}_{oc}_{nh}")
                        for cc in range(CC):
                            nc.tensor.matmul(
                                pj,
                                lhsT=wpack[:, o_ow + cc * C + oc * 128:
                                           o_ow + cc * C + oc * 128 + 128],
                                rhs=d["A"][:, cc, nh * 512:(nh + 1) * 512],
                                start=(cc == 0), stop=(cc == CC - 1))
                        cmb = xpool.tile([128, 512], F32, tag="cmb",
                                         name=f"cmb{img}_{oc}_{nh}")
                        nc.vector.scalar_tensor_tensor(
                            out=cmb, in0=pj, scalar=0.0,
                            in1=c2xs[img][:, oc, nh * 512:(nh + 1) * 512],
                            op0=ALU.add, op1=ALU.add)
                        osb = xpool.tile([128, 512], F32, tag="osb",
                                         name=f"osb{img}_{oc}_{nh}")
                        nc.vector.tensor_scalar(osb, cmb, 0.0, None, ALU.max)
                        nc.sync.dma_start(
                            out=out_d[img, oc, :, nh * 512:(nh + 1) * 512], in_=osb)
                    push(1024, fn)

        # attention sub-blocks: one (head, ncb) group per slot, lagged one
        # full head so all 8 pt tiles of the head exist. Groups are strictly
        # SEQUENTIAL within the psum bank (ncb outer, mc inner): TRN2's
        # start=True lazily re-arms the whole 2KB zero region, so interleaved
        # per-group starts would wipe other groups' partial accumulations.
        attn_subs = []
        at_tiles = {}

        def make_attn_subs(img, h, pts):
            d = qkv_tiles[img]

            def mk(ncb):
                def fn():
                    if ncb == 0:
                        at_tiles[(img, h)] = ps_at.tile(
                            [128, NCH * DAUG], F32, tag="at", name=f"at{img}_{h}")
                    at = at_tiles[(img, h)]
                    for mc in range(MC):
                        nc.tensor.matmul(
                            at[:, ncb * DAUG:(ncb + 1) * DAUG],
                            lhsT=pts[mc][:, ncb * 128:(ncb + 1) * 128],
                            rhs=d["vaug"][:, mc, h, :],
                            start=(mc == 0), stop=(mc == MC - 1),
                            skip_group_check=True)
                    if ncb == NCH - 1:
                        emit_head_norm(img, h, at)
                        del at_tiles[(img, h)]
                        if h == 3:
                            push_transp(img, 0)
                        if h == 7:
                            push_transp(img, 1)
                            push_proj(img)
                return fn

            for ncb in range(NCH):
                attn_subs.append(mk(ncb))

        def head_loop(img):
            d = qkv_tiles[img]
            q_sb, k_sb = d["q"], d["k"]
            for h in range(HEADS):
                hp, cch = 32 * (h % 4), h // 4
                pts = []
                for mc in range(MC):
                    sc = ps_sc.tile([128, N], F32, tag="sc",
                                    name=f"sc{img}_{h}_{mc}")
                    for nh in range(2):
                        nc.tensor.matmul(
                            sc[:, nh * 512:(nh + 1) * 512],
                            lhsT=k_sb[hp:hp + 32, cch, mc * 128:(mc + 1) * 128],
                            rhs=q_sb[hp:hp + 32, cch, nh * 512:(nh + 1) * 512],
                            start=True, stop=True, tile_position=(hp, 0))
                    pt = ptpool.tile([128, N], BF16, tag="pt",
                                     name=f"pt{img}_{h}_{mc}")
                    nc.scalar.activation(pt, sc, AF.Exp)
                    pts.append(pt)
                    if attn_subs:
                        attn_subs.pop(0)()
                    pop_fill()
                make_attn_subs(img, h, pts)

        # ================= emission =================
        xload_dma(0)
        nc.sync.dma_start(out=wpack[:, o_q:], in_=wp_d[:, o_q:])
        nc.sync.dma_start(out=vecs, in_=vec_d[:])
        nc.sync.dma_start(out=wpack[:, :o_q], in_=wp_d[:, :o_q])
        xload_dma(1)
        xcast(0)

        qkv_alloc(0)
        conv_alloc(0)
        qk_chunk(0, 0, "q")
        qk_chunk(0, 0, "k")
        push(2048, lambda: v_chunk(0, 0), front=True)
        push(2048, lambda: v_chunk(0, 1))
        push(2048, lambda: qk_chunk(0, 1, "q"))
        push(2048, lambda: qk_chunk(0, 1, "k"))
        push(0, lambda: xcast(1))
        push_conv_all(0)
        # image 1 prep as filler inside image 0's slots
        push(0, lambda: (qkv_alloc(1), conv_alloc(1)) and None)
        push(2048, lambda: qk_chunk(1, 0, "q"))
        push(2048, lambda: qk_chunk(1, 0, "k"))
        push(2048, lambda: v_chunk(1, 0))
        push(2048, lambda: v_chunk(1, 1))
        push(2048, lambda: qk_chunk(1, 1, "q"))
        push(2048, lambda: qk_chunk(1, 1, "k"))

        head_loop(0)
        push_conv_all(1)
        head_loop(1)
        while attn_subs:
            attn_subs.pop(0)()
        drain_queue()

    nc.finalize()
    return nc


def _prep_inputs(inputs: dict) -> list[dict]:
    bf = ml_dtypes.bfloat16
    x = np.asarray(inputs["x"], dtype=np.float32)
    f32 = lambda k: np.asarray(inputs[k], dtype=np.float32)
    bn1_inv = f32("bn1_gamma") / np.sqrt(f32("bn1_var") + EPS)
    shift1 = f32("bn1_beta") - f32("bn1_mean") * bn1_inv + f32("conv1_b") * bn1_inv
    w1s = f32("conv1_w") * bn1_inv[:, None, None, None]
    bn2_inv = f32("bn2_gamma") / np.sqrt(f32("bn2_var") + EPS)
    shift2 = f32("bn2_beta") - f32("bn2_mean") * bn2_inv + f32("conv2_b") * bn2_inv
    w2s = f32("conv2_w") * bn2_inv[:, None, None, None]
    sg = 1.0 / (1.0 + np.exp(-float(np.asarray(inputs["gate"]))))
    ow = f32("out_w") * sg
    shiftF = shift2 + sg * f32("out_b") + sg * (f32("out_w") @ f32("v_b"))
    qws = f32("q_w") / np.sqrt(D)
    qbs = f32("q_b") / np.sqrt(D)

    def conv_pack(w):  # [O, I, 3, 3] -> [128, CC*9*CC*128]
        t = w.transpose(1, 2, 3, 0).reshape(CC, 128, 3, 3, CC, 128)
        return t.transpose(1, 0, 2, 3, 4, 5).reshape(128, W1_COLS)

    def pack_T(w):  # [O, C_in] -> [128, CC*C]
        return w.T.reshape(CC, 128, C).transpose(1, 0, 2).reshape(128, QKVO_COLS)

    wpack = np.concatenate(
        [conv_pack(w1s), conv_pack(w2s), pack_T(qws), pack_T(f32("k_w")),
         pack_T(f32("v_w")), pack_T(ow)], axis=1).astype(bf)
    assert wpack.shape == (128, PACK_COLS)

    vecs = np.concatenate(
        [np.stack([shift1.reshape(CC, 128), shiftF.reshape(CC, 128),
                   qbs.reshape(CC, 128)]).reshape(3 * CC, 128).T,
         np.eye(128, dtype=np.float32)], axis=1)
    assert vecs.shape == (128, VEC_COLS)
    shared = {"wpack": np.ascontiguousarray(wpack),
              "vecs": np.ascontiguousarray(vecs.astype(np.float32))}
    in_maps = []
    for core in range(N_CORES):
        xs = x[core * IMGS:(core + 1) * IMGS].reshape(IMGS, CC, 128, N)
        in_maps.append({"x_sh": np.ascontiguousarray(xs), **shared})
    return in_maps


_NC_CACHE = {}


def _get_nc():
    if "nc" not in _NC_CACHE:
        _NC_CACHE["nc"] = build_nc()
    return _NC_CACHE["nc"]


def kernel(**inputs) -> np.ndarray:
    nc = _get_nc()
    in_maps = _prep_inputs(inputs)
    res = run_bass_kernel_spmd(nc, in_maps, core_ids=list(range(N_CORES)))
    outs = [res.results[i]["out_sh"].reshape(IMGS, C, H, W) for i in range(N_CORES)]
    return np.concatenate(outs, axis=0)
